# revision 36
# baseline (speedup 1.0000x reference)
"""GQA attention block (B=2, N=2048, D=2048, H=16, KV=4) on 8 TRN2 NeuronCores.

Sharding: sequence-parallel with replicated weights. Core c handles batch
b = c//4, query rows [ (c%4)*512 : (c%4+1)*512 ).  Each core computes its
own Q/K/V projections + RoPE for its row block, AllGathers rope'd K and V,
runs full (non-causal, mask==ones) softmax attention for all 16 heads over
its 512 query rows, and applies the output projection, writing its row
slice of the final output (transposed as [f, n]; host transposes back).

Everything crosses the wire and lives in SBUF as bf16 (fp32 PSUM
accumulation).  The operands are bf16-quantized on the wire anyway, so
bf16 matmuls add no error vs f32r-widened ones, they skip every widening
copy, and HW-measured matmul streams run at the same per-column rate for
both dtypes (the separate InstLdweights a 16-bit matmul emits is hidden by
the PE's reorder window).

Schedule:
  * K and V for kv-group pair {0,1} are projected first and leave in ONE
    combined AllGather (CC_A) at ~30us; groups {2,3} follow (CC_B).  The
    attention runs in two 8-head waves gated on CC_A / CC_B so the
    collectives (~15us constant + bytes/40GBps each, serialized on the
    collective engine) hide under Q-projection + wave-A compute.
  * The softmax denominator is OFF the PE: exp tiles accumulate
    elementwise on DVE into an fp16 accumulator (2-byte operands get the
    DVE fast path), and one [128,1] ones-matmul per head does the final
    partition reduction (8k PE cycles vs 131k for all-PE denominators).
  * exp runs on [128,1024] PSUM pairs (two score tiles per activation
    instruction) to halve the ACT per-instruction overhead, and each wave
    processes TWO heads interleaved so one head's exp round-trip latency
    is covered by the other head's matmuls.
  * The out-projection is interleaved per wave: wave A's partial
    accumulates into an SBUF f32 tile (filling the PE while ACT drains
    wave-A exps and CC_B lands); wave B's pass fuses (psum + bias) +
    partial on DVE and streams the result out.
  * Queue discipline matters more than engine balance: wo prefetch goes on
    the SP HWDGE queue (nothing CC-gated lives there), pair-1 fetches are
    emitted BETWEEN the waves on the Pool queue so their CC_B wait only
    blocks wave-B work, and tc.tile_wait_until hints keep the tile
    scheduler from committing CC-gated instructions ahead of ready wave-A
    work in the in-order engine queues (worth ~50us).
  * V-projection PSUM tiles get a full bank per key-subtile: matmul
    start=True resets the whole 2KB bank, so two 256-wide accumulations
    packed into one bank corrupt each other.
"""

import numpy as np
import ml_dtypes

from concourse import bacc, tile, mybir
from concourse import bass_utils

F32 = mybir.dt.float32
F32R = mybir.dt.float32r
F16 = mybir.dt.float16
BF16 = mybir.dt.bfloat16

P = 128
B, N, D = 2, 2048, 2048
H, HKV, HD = 16, 4, 128
NL = 512          # local query rows per core
ND = D // P       # 16 d-tiles
NKJ = N // P      # 16 key tiles
NFI = D // P      # 16 output-feature tiles
SCALE = 1.0 / np.sqrt(HD)
N_CORES = 8

_CACHE = {}


def _emit(nc, tc, ext, consts, x, single_core=False, stop_after=None):
    """Emit one full forward pass; all tile names prefixed with `x`."""
    (xt_ext, wq_ext, wkv01_ext, wkv23_ext, wo_ext, bias_ext, cos_ext,
     sin_ext, outt_ext) = ext
    (ones_kj_dram,) = consts

    with tc.tile_pool(name=f"{x}const", bufs=1) as cpool, \
         tc.tile_pool(name=f"{x}qr", bufs=1) as qrpool, \
         tc.tile_pool(name=f"{x}fix", bufs=1) as fxpool, \
         tc.tile_pool(name=f"{x}rope", bufs=5) as rpool, \
         tc.tile_pool(name=f"{x}dram", bufs=1, space="DRAM") as dpool, \
         nc.allow_low_precision("f32r matmuls; accum f32"):

        ones_kj = cpool.tile([P, 1], F16, name=f"{x}ones_kj", tag="ones_kj")
        cos_sb = cpool.tile([P, NL], F32, name=f"{x}cos_sb", tag="cos_sb")
        sin_sb = cpool.tile([P, NL], F32, name=f"{x}sin_sb", tag="sin_sb")
        bias_sb = cpool.tile([P, NFI], F32, name=f"{x}bias_sb", tag="bias_sb")

        # combined K+V payloads, one per kv-group pair
        ag_in = [dpool.tile([P, 2048], BF16, name=f"{x}ag{p}_in",
                            tag=f"ag{p}_in") for p in range(2)]
        ag_out = [dpool.tile([4, P, 2048], BF16, name=f"{x}ag{p}_out",
                             tag=f"ag{p}_out") for p in range(2)]

        nc.scalar.dma_start(out=cos_sb[:], in_=cos_ext[:])
        nc.scalar.dma_start(out=sin_sb[:], in_=sin_ext[:])
        nc.scalar.dma_start(out=ones_kj[:], in_=ones_kj_dram.ap())
        nc.scalar.dma_start(out=bias_sb[:], in_=bias_ext[:])

        def rope(dst, src_ps, nm):
            """dst[128,NL] = rope(src_ps[PSUM f32 128,NL]).

            ACT evicts PSUM twice: straight (ev) and half-swapped with the
            second half negated (sw); DVE then does
            y = ev*[cos;cos] + sw*[sin;sin] (3 ops).
            """
            ev = rpool.tile([P, NL], F32, name=f"{x}{nm}_ev", tag="ropet")
            nc.scalar.copy(out=ev[:], in_=src_ps[:])
            sw = rpool.tile([P, NL], F32, name=f"{x}{nm}_sw", tag="ropet")
            nc.scalar.copy(out=sw[0:64, :], in_=src_ps[64:128, :])
            nc.scalar.mul(out=sw[64:128, :], in_=src_ps[0:64, :], mul=-1.0)
            t = rpool.tile([P, NL], F32, name=f"{x}{nm}_t", tag="ropet")
            nc.vector.tensor_tensor(out=t[:], in0=ev[:], in1=cos_sb[:],
                                    op=mybir.AluOpType.mult)
            u = rpool.tile([P, NL], F32, name=f"{x}{nm}_u", tag="ropet")
            nc.vector.tensor_tensor(out=u[:], in0=sw[:], in1=sin_sb[:],
                                    op=mybir.AluOpType.mult)
            nc.vector.tensor_tensor(out=dst[:], in0=t[:], in1=u[:],
                                    op=mybir.AluOpType.add)

        qr_sb = [qrpool.tile([P, NL], BF16, name=f"{x}qr{h}", tag=f"qr{h}")
                 for h in range(H)]

        # gathered K,V stay live through attention; one tile per group pair
        # so wave A never picks up a dependency on the CC_B-gated fetches.
        # kt layout [hd, (j, gi, key)]; vt layout [key, (j, gi, hd)]:
        # both use offset j*1024 + gi*512 + u*128 for key-tile kj=(j,u).
        with tc.tile_pool(name=f"{x}kv", bufs=1) as kvpool:
            kt_sb = [kvpool.tile([P, 4 * 1024], BF16, name=f"{x}kt{p}",
                                 tag=f"kt{p}") for p in range(2)]
            vt_sb = [kvpool.tile([P, 4 * 1024], BF16, name=f"{x}vt{p}",
                                 tag=f"vt{p}") for p in range(2)]

            with tc.tile_pool(name=f"{x}xt", bufs=1) as xpool, \
                 tc.tile_pool(name=f"{x}stage", bufs=1) as stpool:
                xt_sb = xpool.tile([P, ND * NL], BF16, name=f"{x}xt",
                                   tag="xt")

                def xs(dt):
                    return xt_sb[:, dt * NL:(dt + 1) * NL]

                kv_stage = [stpool.tile([P, 2048], BF16, name=f"{x}kvs{p}",
                                        tag=f"kvs{p}") for p in range(2)]

                # ---- K+V projection per group pair + its AllGather ----
                with tc.tile_pool(name=f"{x}wkv", bufs=3) as kvwpool, \
                     tc.tile_pool(name=f"{x}ppkv", bufs=1,
                                  space="PSUM") as ppkv:
                    for pair in range(2):
                        wext = wkv01_ext if pair == 0 else wkv23_ext
                        kab = ppkv.tile([P, 2 * NL], F32,
                                        name=f"{x}kab{pair}", tag="kab",
                                        bufs=2)
                        # one full PSUM bank per key-subtile: a matmul
                        # start=True reset is bank-wide, so packing two
                        # 256-wide accumulations into one bank corrupts
                        # the earlier one's first dt contribution.
                        vts = [ppkv.tile([P, NL], F32,
                                         name=f"{x}vab{pair}_{t}",
                                         tag="vab4", bufs=4)
                               for t in range(4)]
                        for c in range(ND // 2):
                            if pair == 0:
                                nc.sync.dma_start(
                                    out=xt_sb[:, c * 1024:(c + 1) * 1024],
                                    in_=xt_ext[2 * c:2 * c + 2]
                                    .transpose([1, 0, 2]))
                            wkv = kvwpool.tile([P, 1024], BF16,
                                               name=f"{x}wkv{pair}_{c}",
                                               tag="wkv")
                            nc.sync.dma_start(
                                out=wkv[:],
                                in_=wext[2 * c:2 * c + 2].transpose([1, 0, 2]))
                            for i in range(2):
                                dt = 2 * c + i
                                for gi in range(2):
                                    nc.tensor.matmul(
                                        kab[:, gi * NL:(gi + 1) * NL],
                                        wkv[:, i * NL + gi * P:
                                            i * NL + (gi + 1) * P],
                                        xs(dt),
                                        start=(dt == 0), stop=(dt == ND - 1))
                                for t in range(4):
                                    nc.tensor.matmul(
                                        vts[t][:, 0:256],
                                        xs(dt)[:, t * P:(t + 1) * P],
                                        wkv[:, i * NL + 256:i * NL + NL],
                                        start=(dt == 0), stop=(dt == ND - 1))
                        # rope K -> stage cols [0:1024); V -> [1024:2048)
                        for gi in range(2):
                            rope(kv_stage[pair][:, gi * NL:(gi + 1) * NL],
                                 kab[:, gi * NL:(gi + 1) * NL],
                                 f"k{pair}_{gi}")
                        for t in range(4):
                            for gi in range(2):
                                nc.vector.tensor_copy(
                                    out=kv_stage[pair][
                                        :, 1024 + gi * NL + t * P:
                                        1024 + gi * NL + (t + 1) * P],
                                    in_=vts[t][:, gi * P:(gi + 1) * P])
                        nc.gpsimd.dma_start(out=ag_in[pair][:],
                                            in_=kv_stage[pair][:])
                        if single_core:
                            nc.gpsimd.dma_start(out=ag_out[pair][0],
                                                in_=ag_in[pair][:])
                        else:
                            nc.gpsimd.collective_compute(
                                "AllGather",
                                mybir.AluOpType.bypass,
                                ins=[ag_in[pair][:]],
                                outs=[ag_out[pair][:]],
                                replica_groups=[[0, 1, 2, 3], [4, 5, 6, 7]],
                            )

                # ---- Q projection + RoPE (overlaps the collectives) ----
                with tc.tile_pool(name=f"{x}wq", bufs=3) as wqpool, \
                     tc.tile_pool(name=f"{x}ppq", bufs=1,
                                  space="PSUM") as ppq:
                    for hg in range(4):
                        qa = ppq.tile([P, 2 * NL], F32, name=f"{x}qa{hg}",
                                      tag="qp", bufs=4)
                        qb = ppq.tile([P, 2 * NL], F32, name=f"{x}qb{hg}",
                                      tag="qp", bufs=4)
                        psq = [qa[:, 0:NL], qa[:, NL:2 * NL],
                               qb[:, 0:NL], qb[:, NL:2 * NL]]
                        for dp in range(ND // 2):
                            wt = wqpool.tile([P, 1024], BF16,
                                             name=f"{x}wqb{hg}_{dp}",
                                             tag="wqb", bufs=4)
                            nc.sync.dma_start(
                                out=wt[:],
                                in_=wq_ext[hg, 2 * dp:2 * dp + 2]
                                .transpose([1, 0, 2]))
                            for i in range(2):
                                dt = 2 * dp + i
                                for hh in range(4):
                                    nc.tensor.matmul(
                                        psq[hh][:],
                                        wt[:, i * NL + hh * P:
                                           i * NL + (hh + 1) * P],
                                        xs(dt),
                                        start=(dt == 0),
                                        stop=(dt == ND - 1))
                        for hh in range(4):
                            h = hg * 4 + hh
                            rope(qr_sb[h], psq[hh], f"q{h}")

            # ---- wo prefetch (Pool SWDGE) + gathered K/V fetch/widen ----
            wo_bf = []
            with tc.tile_pool(name=f"{x}wo", bufs=1) as wopool, \
                 tc.tile_pool(name=f"{x}exps", bufs=3) as epool, \
                 tc.tile_pool(name=f"{x}acc", bufs=4) as apool, \
                 tc.tile_pool(name=f"{x}no", bufs=1) as nopool, \
                 tc.tile_pool(name=f"{x}oacc", bufs=1) as oapool, \
                 tc.tile_pool(name=f"{x}outsb", bufs=2) as opool:

                # fetch gathered K/V with SWDGE cast-DMAs (bf16 in DRAM
                # -> f32r in SBUF): no staging tiles, no widen ops.  Pair 0
                # is fetched before the attention; pair 1 is emitted BETWEEN
                # the waves so its CC_B wait at the Pool queue head only
                # blocks wave-B work.
                def fetch_pair(pair):
                    if single_core:
                        for j in range(4):
                            o = j * 1024
                            nc.gpsimd.dma_start(
                                out=kt_sb[pair][:, o:o + 1024],
                                in_=ag_out[pair][0][:, 0:1024])
                            nc.gpsimd.dma_start(
                                out=vt_sb[pair][:, o:o + 1024],
                                in_=ag_out[pair][0][:, 1024:2048])
                        return
                    # one multi-dim DMA per tensor: [4, P, 1024] -> [P, 4*1024]
                    nc.gpsimd.dma_start(
                        out=kt_sb[pair][:],
                        in_=ag_out[pair][:, :, 0:1024].transpose([1, 0, 2]))
                    nc.gpsimd.dma_start(
                        out=vt_sb[pair][:],
                        in_=ag_out[pair][:, :, 1024:2048].transpose([1, 0, 2]))

                with tc.tile_wait_until(0.107):
                    fetch_pair(0)

                for fi in range(NFI):
                    wt = wopool.tile([P, H * P], BF16, name=f"{x}wob{fi}",
                                     tag="wob", bufs=16)
                    # SP HWDGE: nothing CC-gated lives there now (pair-1
                    # fetches issue from Pool between the waves), so these
                    # prefetches land long before the wave-A out-proj.
                    nc.sync.dma_start(out=wt[:], in_=wo_ext[fi])
                    wo_bf.append(wt)

                out_acc = oapool.tile([P, NFI * NL], F32,
                                      name=f"{x}oacc", tag="oacc")

                def kslice(g, kj):
                    j, u = divmod(kj, 4)
                    o = j * 1024 + (g % 2) * NL + u * P
                    return kt_sb[g // 2][:, o:o + P]

                def vslice(g, kj):
                    j, u = divmod(kj, 4)
                    o = j * 1024 + (g % 2) * NL + u * P
                    return vt_sb[g // 2][:, o:o + P]

                # ---- attention: two 8-head waves ----
                no_sb = [None] * H
                with tc.tile_pool(name=f"{x}ppatt", bufs=1,
                                  space="PSUM") as pp:
                    for pair in range(2):
                        if pair == 1:
                            with tc.tile_wait_until(0.178):
                                fetch_pair(1)
                        heads = [2 * pair + (hh % 2) + 4 * (hh // 2)
                                 for hh in range(8)]
                        # two heads run interleaved: each head's exp
                        # round-trip latency is covered by the OTHER head's
                        # matmuls (HW semaphore hops are slower than the
                        # model, so per-head lag-1 alone stalls the PE).
                        for hi in range(0, 8, 2):
                            hpair = heads[hi:hi + 2]
                            av_ps = {}
                            acc = {}
                            e_tiles = {}
                            for h in hpair:
                                av_ps[h] = pp.tile([P, NL], F32,
                                                   name=f"{x}av{h}",
                                                   tag="av", bufs=2)
                                acc[h] = apool.tile([P, NL], F16,
                                                    name=f"{x}acc{h}",
                                                    tag="acc")
                            for step in range(9):
                                if step < 8:
                                    for h in hpair:
                                        g = h % HKV
                                        s_ps = pp.tile(
                                            [P, 2 * NL], F32,
                                            name=f"{x}s{h}_{step}",
                                            tag="sc", bufs=2)
                                        for i in range(2):
                                            kj = 2 * step + i
                                            nc.tensor.matmul(
                                                s_ps[:, i * NL:(i + 1) * NL],
                                                kslice(g, kj), qr_sb[h][:],
                                                start=True, stop=True)
                                        e_sb = epool.tile(
                                            [P, 2 * NL], BF16,
                                            name=f"{x}e{h}_{step}",
                                            tag="exp", bufs=8)
                                        nc.scalar.activation(
                                            e_sb[:], s_ps[:],
                                            mybir.ActivationFunctionType.Exp,
                                            scale=float(SCALE))
                                        e_tiles[(h, step)] = e_sb
                                if step >= 1:
                                    p2 = step - 1
                                    for h in hpair:
                                        g = h % HKV
                                        e_sb = e_tiles.pop((h, p2))
                                        for i in range(2):
                                            kj = 2 * p2 + i
                                            esl = e_sb[:, i * NL:(i + 1) * NL]
                                            nc.tensor.matmul(
                                                av_ps[h][:], vslice(g, kj),
                                                esl,
                                                start=(kj == 0),
                                                stop=(kj == NKJ - 1))
                                            # denominator on DVE (fp16 acc,
                                            # 2-byte fast mode); one final
                                            # ones-matmul per head reduces
                                            # across partitions.
                                            if kj == 0:
                                                nc.vector.tensor_copy(
                                                    out=acc[h][:], in_=esl)
                                            else:
                                                nc.vector.tensor_tensor(
                                                    out=acc[h][:],
                                                    in0=acc[h][:],
                                                    in1=esl,
                                                    op=mybir.AluOpType.add)
                            for h in hpair:
                                den_ps = pp.tile([1, NL], F32,
                                                 name=f"{x}den{h}",
                                                 tag="den", bufs=1)
                                nc.tensor.matmul(den_ps[:], ones_kj[:],
                                                 acc[h][:],
                                                 start=True, stop=True)
                                recip = fxpool.tile([1, NL], F32,
                                                    name=f"{x}rc{h}",
                                                    tag="recip", bufs=2)
                                nc.vector.reciprocal(out=recip[:],
                                                     in_=den_ps[:])
                                bc_sb = fxpool.tile([P, NL], F32,
                                                    name=f"{x}bcs{h}",
                                                    tag="bcs", bufs=2)
                                nc.gpsimd.partition_broadcast(bc_sb[:],
                                                              recip[:])
                                no = nopool.tile([P, NL], BF16,
                                                 name=f"{x}no{h}",
                                                 tag=f"no{h}")
                                nc.vector.tensor_tensor(
                                    out=no[:], in0=av_ps[h][:],
                                    in1=bc_sb[:],
                                    op=mybir.AluOpType.mult)
                                no_sb[h] = no

                        # partial out-projection over this wave's heads:
                        # fills the PE while ACT drains this wave's exps and
                        # the next collective lands.
                        for fi in range(NFI):
                            ps = pp.tile([P, NL], F32,
                                         name=f"{x}po{pair}_{fi}",
                                         tag="pso", bufs=1)
                            for h in heads:
                                nc.tensor.matmul(
                                    ps[:], wo_bf[fi][:, h * P:(h + 1) * P],
                                    no_sb[h][:],
                                    start=(h == heads[0]),
                                    stop=(h == heads[-1]))
                            oa = out_acc[:, fi * NL:(fi + 1) * NL]
                            if pair == 0:
                                nc.vector.tensor_copy(out=oa, in_=ps[:])
                            else:
                                o_sb = opool.tile([P, NL], BF16,
                                                  name=f"{x}o{fi}", tag="osb")
                                nc.vector.scalar_tensor_tensor(
                                    out=o_sb[:], in0=ps[:],
                                    scalar=bias_sb[:, fi:fi + 1],
                                    in1=oa,
                                    op0=mybir.AluOpType.add,
                                    op1=mybir.AluOpType.add)
                                nc.scalar.dma_start(out=outt_ext[fi],
                                                    in_=o_sb[:])

                if stop_after == "attn":
                    nc.sync.dma_start(out=outt_ext[0],
                                      in_=no_sb[0][:].bitcast(F32))
                    return

def build_program(reps=1, single_core=False):
    nc = bacc.Bacc("TRN2", target_bir_lowering=False, debug=False,
                   num_devices=1 if single_core else N_CORES)

    ext = (
        nc.dram_tensor("xt", [ND, P, NL], BF16,
                       kind="ExternalInput").ap(),
        nc.dram_tensor("wqtt", [4, ND, P, NL], BF16,
                       kind="ExternalInput").ap(),
        nc.dram_tensor("wkv01t", [ND, P, NL], BF16,
                       kind="ExternalInput").ap(),
        nc.dram_tensor("wkv23t", [ND, P, NL], BF16,
                       kind="ExternalInput").ap(),
        nc.dram_tensor("wott", [NFI, P, H * P], BF16,
                       kind="ExternalInput").ap(),
        nc.dram_tensor("biast", [P, NFI], F32, kind="ExternalInput").ap(),
        nc.dram_tensor("cost", [P, NL], F32, kind="ExternalInput").ap(),
        nc.dram_tensor("sint", [P, NL], F32, kind="ExternalInput").ap(),
        nc.dram_tensor("outt", [NFI, P, NL], BF16,
                       kind="ExternalOutput").ap(),
    )
    consts = (
        nc.inline_tensor(np.ones((P, 1), np.float16), name="ones_kj_c"),
    )

    with tile.TileContext(nc) as tc:
        for r in range(reps):
            _emit(nc, tc, ext, consts, f"r{r}_" if reps > 1 else "",
                  single_core=single_core)

    nc.compile()
    return nc


def shard_inputs(x, cos, sin, wq, wkv, wo_w, wo_b):
    """Host-side prep: transpose/tile everything into DMA-friendly layouts."""
    x = np.asarray(x, np.float32)
    cos = np.asarray(cos, np.float32)
    sin = np.asarray(sin, np.float32)
    wq = np.asarray(wq, np.float32)
    wkv = np.asarray(wkv, np.float32)
    wo_w = np.asarray(wo_w, np.float32)
    wo_b = np.asarray(wo_b, np.float32)

    wqT = np.ascontiguousarray(wq.T)                      # [d, e]
    # tiles [hg, dt, 128, 512]
    wqtt = np.ascontiguousarray(
        wqT.reshape(ND, P, 4, NL).transpose(2, 0, 1, 3)).astype(
            ml_dtypes.bfloat16)
    wkvT = wkv.T                                          # [d, 1024]
    wk, wv = wkvT[:, 0:512], wkvT[:, 512:1024]
    # per pair: [d, 512] = [K pair (2*128) | V pair (2*128)]
    wkv01 = np.ascontiguousarray(
        np.concatenate([wk[:, 0:256], wv[:, 0:256]], axis=1)
    ).reshape(ND, P, NL).astype(ml_dtypes.bfloat16)
    wkv23 = np.ascontiguousarray(
        np.concatenate([wk[:, 256:512], wv[:, 256:512]], axis=1)
    ).reshape(ND, P, NL).astype(ml_dtypes.bfloat16)
    woT = wo_w.T                                          # [e, f]
    # [fi, a, h, b]: per fi a contiguous [128, 2048] block
    wott = np.ascontiguousarray(
        woT.reshape(H, P, NFI, P).transpose(2, 1, 0, 3)
    ).reshape(NFI, P, H * P).astype(ml_dtypes.bfloat16)
    biast = np.ascontiguousarray(wo_b.reshape(NFI, P).T)  # [128, 16] f32

    in_maps = []
    for c in range(N_CORES):
        b, blk = divmod(c, 4)
        r0 = blk * NL
        xt = np.ascontiguousarray(x[b, r0:r0 + NL, :].T).reshape(
            ND, P, NL).astype(ml_dtypes.bfloat16)
        cosT = cos[0, r0:r0 + NL, 0, :].T                 # [64, n]
        sinT = sin[0, r0:r0 + NL, 0, :].T
        cost = np.ascontiguousarray(np.vstack([cosT, cosT]))   # [128, n]
        sint = np.ascontiguousarray(np.vstack([sinT, sinT]))
        in_maps.append({
            "xt": xt, "wqtt": wqtt, "wkv01t": wkv01, "wkv23t": wkv23,
            "wott": wott, "biast": biast, "cost": cost, "sint": sint,
        })
    return in_maps


def assemble_output(results):
    out = np.empty((B, N, D), np.float32)
    for c in range(N_CORES):
        b, blk = divmod(c, 4)
        r0 = blk * NL
        # outt [NFI, P, NL] -> [d, n] -> transpose
        out[b, r0:r0 + NL, :] = results[c]["outt"].reshape(
            D, NL).astype(np.float32).T
    return out


def get_program(reps=1):
    key = ("nc", reps)
    if key not in _CACHE:
        _CACHE[key] = build_program(reps)
    return _CACHE[key]


def kernel(x, cos, sin, attn_mask, wq, wkv, wo_w, wo_b):
    # attn_mask is all-ones by construction (fill spec); ignored.
    nc = get_program()
    in_maps = shard_inputs(x, cos, sin, wq, wkv, wo_w, wo_b)
    res = bass_utils.run_bass_kernel_spmd(
        nc, in_maps, core_ids=list(range(N_CORES)))
    return assemble_output(res.results)


# revision 40
# speedup vs baseline: 1.0137x; 1.0137x over previous
"""GQA attention block (B=2, N=2048, D=2048, H=16, KV=4) on 8 TRN2 NeuronCores.

Sharding: sequence-parallel with replicated weights. Core c handles batch
b = c//4, query rows [ (c%4)*512 : (c%4+1)*512 ).  Each core computes its
own Q/K/V projections + RoPE for its row block, AllGathers rope'd K and V,
runs full (non-causal, mask==ones) softmax attention for all 16 heads over
its 512 query rows, and applies the output projection, writing its row
slice of the final output (transposed as [f, n]; host transposes back).

Everything crosses the wire and lives in SBUF as bf16 (fp32 PSUM
accumulation).  The operands are bf16-quantized on the wire anyway, so
bf16 matmuls add no error vs f32r-widened ones, they skip every widening
copy, and HW-measured matmul streams run at the same per-column rate for
both dtypes (the separate InstLdweights a 16-bit matmul emits is hidden by
the PE's reorder window).

Schedule:
  * K and V for kv-group pair {0,1} are projected first and leave in ONE
    combined AllGather (CC_A) at ~30us; groups {2,3} follow (CC_B).  The
    attention runs in two 8-head waves gated on CC_A / CC_B so the
    collectives (~15us constant + bytes/40GBps each, serialized on the
    collective engine) hide under Q-projection + wave-A compute.
  * The softmax denominator is OFF the PE: exp tiles accumulate
    elementwise on DVE into an fp16 accumulator (2-byte operands get the
    DVE fast path), and one [128,1] ones-matmul per head does the final
    partition reduction (8k PE cycles vs 131k for all-PE denominators).
  * exp runs on [128,1024] PSUM pairs (two score tiles per activation
    instruction) to halve the ACT per-instruction overhead, and each wave
    processes TWO heads interleaved so one head's exp round-trip latency
    is covered by the other head's matmuls.
  * The out-projection is interleaved per wave: wave A's partial
    accumulates into an SBUF f32 tile (filling the PE while ACT drains
    wave-A exps and CC_B lands); wave B's pass fuses (psum + bias) +
    partial on DVE and streams the result out.
  * Queue discipline matters more than engine balance: wo prefetch goes on
    the SP HWDGE queue (nothing CC-gated lives there), pair-1 fetches are
    emitted BETWEEN the waves on the Pool queue so their CC_B wait only
    blocks wave-B work, and tc.tile_wait_until hints keep the tile
    scheduler from committing CC-gated instructions ahead of ready wave-A
    work in the in-order engine queues (worth ~50us).
  * V-projection PSUM tiles get a full bank per key-subtile: matmul
    start=True resets the whole 2KB bank, so two 256-wide accumulations
    packed into one bank corrupt each other.
"""

import numpy as np
import ml_dtypes

from concourse import bacc, tile, mybir
from concourse import bass_utils

F32 = mybir.dt.float32
F32R = mybir.dt.float32r
F16 = mybir.dt.float16
BF16 = mybir.dt.bfloat16

P = 128
B, N, D = 2, 2048, 2048
H, HKV, HD = 16, 4, 128
NL = 512          # local query rows per core
ND = D // P       # 16 d-tiles
NKJ = N // P      # 16 key tiles
NFI = D // P      # 16 output-feature tiles
SCALE = 1.0 / np.sqrt(HD)
N_CORES = 8

_CACHE = {}


def _emit(nc, tc, ext, consts, x, single_core=False, stop_after=None,
          t0=0.0):
    """Emit one full forward pass; all tile names prefixed with `x`."""
    (xt_ext, wq_ext, wkv01_ext, wkv23_ext, wo_ext, bias_ext, cos_ext,
     sin_ext, outt_ext) = ext
    (ones_kj_dram,) = consts

    with tc.tile_pool(name=f"{x}const", bufs=1) as cpool, \
         tc.tile_pool(name=f"{x}qr", bufs=1) as qrpool, \
         tc.tile_pool(name=f"{x}fix", bufs=1) as fxpool, \
         tc.tile_pool(name=f"{x}rope", bufs=5) as rpool, \
         tc.tile_pool(name=f"{x}dram", bufs=1, space="DRAM") as dpool, \
         nc.allow_low_precision("f32r matmuls; accum f32"):

        ones_kj = cpool.tile([P, 1], F16, name=f"{x}ones_kj", tag="ones_kj")
        cos_sb = cpool.tile([P, NL], F32, name=f"{x}cos_sb", tag="cos_sb")
        sin_sb = cpool.tile([P, NL], F32, name=f"{x}sin_sb", tag="sin_sb")
        bias_sb = cpool.tile([P, NFI], F32, name=f"{x}bias_sb", tag="bias_sb")

        # combined K+V payloads, one per kv-group pair
        ag_in = [dpool.tile([P, 2048], BF16, name=f"{x}ag{p}_in",
                            tag=f"ag{p}_in") for p in range(2)]
        ag_out = [dpool.tile([4, P, 2048], BF16, name=f"{x}ag{p}_out",
                             tag=f"ag{p}_out") for p in range(2)]

        nc.scalar.dma_start(out=cos_sb[:], in_=cos_ext[:])
        nc.scalar.dma_start(out=sin_sb[:], in_=sin_ext[:])
        nc.scalar.dma_start(out=ones_kj[:], in_=ones_kj_dram.ap())
        nc.scalar.dma_start(out=bias_sb[:], in_=bias_ext[:])

        def rope(dst, src_ps, nm):
            """dst[128,NL] = rope(src_ps[PSUM f32 128,NL]).

            ACT evicts PSUM twice: straight (ev) and half-swapped with the
            second half negated (sw); DVE then does
            y = ev*[cos;cos] + sw*[sin;sin] (3 ops).
            """
            ev = rpool.tile([P, NL], F32, name=f"{x}{nm}_ev", tag="ropet")
            nc.scalar.copy(out=ev[:], in_=src_ps[:])
            sw = rpool.tile([P, NL], F32, name=f"{x}{nm}_sw", tag="ropet")
            nc.scalar.copy(out=sw[0:64, :], in_=src_ps[64:128, :])
            nc.scalar.mul(out=sw[64:128, :], in_=src_ps[0:64, :], mul=-1.0)
            t = rpool.tile([P, NL], F32, name=f"{x}{nm}_t", tag="ropet")
            nc.vector.tensor_tensor(out=t[:], in0=ev[:], in1=cos_sb[:],
                                    op=mybir.AluOpType.mult)
            u = rpool.tile([P, NL], F32, name=f"{x}{nm}_u", tag="ropet")
            nc.vector.tensor_tensor(out=u[:], in0=sw[:], in1=sin_sb[:],
                                    op=mybir.AluOpType.mult)
            nc.vector.tensor_tensor(out=dst[:], in0=t[:], in1=u[:],
                                    op=mybir.AluOpType.add)

        qr_sb = [qrpool.tile([P, NL], BF16, name=f"{x}qr{h}", tag=f"qr{h}")
                 for h in range(H)]

        # gathered K,V stay live through attention; one tile per group pair
        # so wave A never picks up a dependency on the CC_B-gated fetches.
        # kt layout [hd, (j, gi, key)]; vt layout [key, (j, gi, hd)]:
        # both use offset j*1024 + gi*512 + u*128 for key-tile kj=(j,u).
        with tc.tile_pool(name=f"{x}kv", bufs=1) as kvpool:
            kt_sb = [kvpool.tile([P, 4 * 1024], BF16, name=f"{x}kt{p}",
                                 tag=f"kt{p}") for p in range(2)]
            vt_sb = [kvpool.tile([P, 4 * 1024], BF16, name=f"{x}vt{p}",
                                 tag=f"vt{p}") for p in range(2)]

            with tc.tile_pool(name=f"{x}xt", bufs=1) as xpool, \
                 tc.tile_pool(name=f"{x}stage", bufs=1) as stpool:
                xt_sb = xpool.tile([P, ND * NL], BF16, name=f"{x}xt",
                                   tag="xt")

                def xs(dt):
                    return xt_sb[:, dt * NL:(dt + 1) * NL]

                kv_stage = [stpool.tile([P, 2048], BF16, name=f"{x}kvs{p}",
                                        tag=f"kvs{p}") for p in range(2)]

                # ---- K+V projection per group pair + its AllGather ----
                with tc.tile_pool(name=f"{x}wkv", bufs=3) as kvwpool, \
                     tc.tile_pool(name=f"{x}ppkv", bufs=1,
                                  space="PSUM") as ppkv:
                    for pair in range(2):
                        wext = wkv01_ext if pair == 0 else wkv23_ext
                        kab = ppkv.tile([P, 2 * NL], F32,
                                        name=f"{x}kab{pair}", tag="kab",
                                        bufs=2)
                        # one full PSUM bank per key-subtile: a matmul
                        # start=True reset is bank-wide, so packing two
                        # 256-wide accumulations into one bank corrupts
                        # the earlier one's first dt contribution.
                        vts = [ppkv.tile([P, NL], F32,
                                         name=f"{x}vab{pair}_{t}",
                                         tag="vab4", bufs=4)
                               for t in range(4)]
                        for c in range(ND // 2):
                            if pair == 0:
                                nc.sync.dma_start(
                                    out=xt_sb[:, c * 1024:(c + 1) * 1024],
                                    in_=xt_ext[2 * c:2 * c + 2]
                                    .transpose([1, 0, 2]))
                            wkv = kvwpool.tile([P, 1024], BF16,
                                               name=f"{x}wkv{pair}_{c}",
                                               tag="wkv")
                            nc.sync.dma_start(
                                out=wkv[:],
                                in_=wext[2 * c:2 * c + 2].transpose([1, 0, 2]))
                            for i in range(2):
                                dt = 2 * c + i
                                for gi in range(2):
                                    nc.tensor.matmul(
                                        kab[:, gi * NL:(gi + 1) * NL],
                                        wkv[:, i * NL + gi * P:
                                            i * NL + (gi + 1) * P],
                                        xs(dt),
                                        start=(dt == 0), stop=(dt == ND - 1))
                                for t in range(4):
                                    nc.tensor.matmul(
                                        vts[t][:, 0:256],
                                        xs(dt)[:, t * P:(t + 1) * P],
                                        wkv[:, i * NL + 256:i * NL + NL],
                                        start=(dt == 0), stop=(dt == ND - 1))
                        # rope K -> stage cols [0:1024); V -> [1024:2048)
                        for gi in range(2):
                            rope(kv_stage[pair][:, gi * NL:(gi + 1) * NL],
                                 kab[:, gi * NL:(gi + 1) * NL],
                                 f"k{pair}_{gi}")
                        for t in range(4):
                            for gi in range(2):
                                nc.vector.tensor_copy(
                                    out=kv_stage[pair][
                                        :, 1024 + gi * NL + t * P:
                                        1024 + gi * NL + (t + 1) * P],
                                    in_=vts[t][:, gi * P:(gi + 1) * P])
                        nc.gpsimd.dma_start(out=ag_in[pair][:],
                                            in_=kv_stage[pair][:])
                        if single_core:
                            nc.gpsimd.dma_start(out=ag_out[pair][0],
                                                in_=ag_in[pair][:])
                        else:
                            nc.gpsimd.collective_compute(
                                "AllGather",
                                mybir.AluOpType.bypass,
                                ins=[ag_in[pair][:]],
                                outs=[ag_out[pair][:]],
                                replica_groups=[[0, 1, 2, 3], [4, 5, 6, 7]],
                            )

                # ---- Q projection + RoPE (overlaps the collectives) ----
                with tc.tile_pool(name=f"{x}wq", bufs=3) as wqpool, \
                     tc.tile_pool(name=f"{x}ppq", bufs=1,
                                  space="PSUM") as ppq:
                    for hg in range(4):
                        qa = ppq.tile([P, 2 * NL], F32, name=f"{x}qa{hg}",
                                      tag="qp", bufs=4)
                        qb = ppq.tile([P, 2 * NL], F32, name=f"{x}qb{hg}",
                                      tag="qp", bufs=4)
                        psq = [qa[:, 0:NL], qa[:, NL:2 * NL],
                               qb[:, 0:NL], qb[:, NL:2 * NL]]
                        for dp in range(ND // 2):
                            wt = wqpool.tile([P, 1024], BF16,
                                             name=f"{x}wqb{hg}_{dp}",
                                             tag="wqb", bufs=4)
                            nc.sync.dma_start(
                                out=wt[:],
                                in_=wq_ext[hg, 2 * dp:2 * dp + 2]
                                .transpose([1, 0, 2]))
                            for i in range(2):
                                dt = 2 * dp + i
                                for hh in range(4):
                                    nc.tensor.matmul(
                                        psq[hh][:],
                                        wt[:, i * NL + hh * P:
                                           i * NL + (hh + 1) * P],
                                        xs(dt),
                                        start=(dt == 0),
                                        stop=(dt == ND - 1))
                        for hh in range(4):
                            h = hg * 4 + hh
                            rope(qr_sb[h], psq[hh], f"q{h}")

            # ---- wo prefetch (Pool SWDGE) + gathered K/V fetch/widen ----
            wo_bf = []
            with tc.tile_pool(name=f"{x}wo", bufs=1) as wopool, \
                 tc.tile_pool(name=f"{x}exps", bufs=3) as epool, \
                 tc.tile_pool(name=f"{x}acc", bufs=4) as apool, \
                 tc.tile_pool(name=f"{x}no", bufs=1) as nopool, \
                 tc.tile_pool(name=f"{x}oacc", bufs=1) as oapool, \
                 tc.tile_pool(name=f"{x}outsb", bufs=2) as opool:

                # fetch gathered K/V with SWDGE cast-DMAs (bf16 in DRAM
                # -> f32r in SBUF): no staging tiles, no widen ops.  Pair 0
                # is fetched before the attention; pair 1 is emitted BETWEEN
                # the waves so its CC_B wait at the Pool queue head only
                # blocks wave-B work.
                def fetch_pair(pair):
                    if single_core:
                        for j in range(4):
                            o = j * 1024
                            nc.gpsimd.dma_start(
                                out=kt_sb[pair][:, o:o + 1024],
                                in_=ag_out[pair][0][:, 0:1024])
                            nc.gpsimd.dma_start(
                                out=vt_sb[pair][:, o:o + 1024],
                                in_=ag_out[pair][0][:, 1024:2048])
                        return
                    # one multi-dim DMA per tensor: [4, P, 1024] -> [P, 4*1024]
                    nc.gpsimd.dma_start(
                        out=kt_sb[pair][:],
                        in_=ag_out[pair][:, :, 0:1024].transpose([1, 0, 2]))
                    nc.gpsimd.dma_start(
                        out=vt_sb[pair][:],
                        in_=ag_out[pair][:, :, 1024:2048].transpose([1, 0, 2]))

                with tc.tile_wait_until(t0 + 0.107):
                    fetch_pair(0)

                for fi in range(NFI):
                    wt = wopool.tile([P, H * P], BF16, name=f"{x}wob{fi}",
                                     tag="wob", bufs=16)
                    # SP HWDGE: nothing CC-gated lives there now (pair-1
                    # fetches issue from Pool between the waves), so these
                    # prefetches land long before the wave-A out-proj.
                    nc.sync.dma_start(out=wt[:], in_=wo_ext[fi])
                    wo_bf.append(wt)

                out_acc = oapool.tile([P, NFI * NL], F32,
                                      name=f"{x}oacc", tag="oacc")

                def kslice(g, kj):
                    j, u = divmod(kj, 4)
                    o = j * 1024 + (g % 2) * NL + u * P
                    return kt_sb[g // 2][:, o:o + P]

                def vslice(g, kj):
                    j, u = divmod(kj, 4)
                    o = j * 1024 + (g % 2) * NL + u * P
                    return vt_sb[g // 2][:, o:o + P]

                # ---- attention: two 8-head waves ----
                no_sb = [None] * H
                with tc.tile_pool(name=f"{x}ppatt", bufs=1,
                                  space="PSUM") as pp:
                    for pair in range(2):
                        if pair == 1:
                            with tc.tile_wait_until(t0 + 0.178):
                                fetch_pair(1)
                        heads = [2 * pair + (hh % 2) + 4 * (hh // 2)
                                 for hh in range(8)]
                        # two heads run interleaved: each head's exp
                        # round-trip latency is covered by the OTHER head's
                        # matmuls (HW semaphore hops are slower than the
                        # model, so per-head lag-1 alone stalls the PE).
                        for hi in range(0, 8, 2):
                            hpair = heads[hi:hi + 2]
                            av_ps = {}
                            acc = {}
                            e_tiles = {}
                            for h in hpair:
                                av_ps[h] = pp.tile([P, NL], F32,
                                                   name=f"{x}av{h}",
                                                   tag="av", bufs=2)
                                acc[h] = apool.tile([P, NL], F16,
                                                    name=f"{x}acc{h}",
                                                    tag="acc")
                            for step in range(9):
                                if step < 8:
                                    for h in hpair:
                                        g = h % HKV
                                        s_ps = pp.tile(
                                            [P, 2 * NL], F32,
                                            name=f"{x}s{h}_{step}",
                                            tag="sc", bufs=2)
                                        for i in range(2):
                                            kj = 2 * step + i
                                            nc.tensor.matmul(
                                                s_ps[:, i * NL:(i + 1) * NL],
                                                kslice(g, kj), qr_sb[h][:],
                                                start=True, stop=True)
                                        e_sb = epool.tile(
                                            [P, 2 * NL], BF16,
                                            name=f"{x}e{h}_{step}",
                                            tag="exp", bufs=8)
                                        nc.scalar.activation(
                                            e_sb[:], s_ps[:],
                                            mybir.ActivationFunctionType.Exp,
                                            scale=float(SCALE))
                                        e_tiles[(h, step)] = e_sb
                                if step >= 1:
                                    p2 = step - 1
                                    for h in hpair:
                                        g = h % HKV
                                        e_sb = e_tiles.pop((h, p2))
                                        for i in range(2):
                                            kj = 2 * p2 + i
                                            esl = e_sb[:, i * NL:(i + 1) * NL]
                                            nc.tensor.matmul(
                                                av_ps[h][:], vslice(g, kj),
                                                esl,
                                                start=(kj == 0),
                                                stop=(kj == NKJ - 1))
                                            # denominator on DVE (fp16 acc,
                                            # 2-byte fast mode); one final
                                            # ones-matmul per head reduces
                                            # across partitions.
                                            if kj == 0:
                                                nc.vector.tensor_copy(
                                                    out=acc[h][:], in_=esl)
                                            else:
                                                nc.vector.tensor_tensor(
                                                    out=acc[h][:],
                                                    in0=acc[h][:],
                                                    in1=esl,
                                                    op=mybir.AluOpType.add)
                            for h in hpair:
                                den_ps = pp.tile([1, NL], F32,
                                                 name=f"{x}den{h}",
                                                 tag="den", bufs=1)
                                nc.tensor.matmul(den_ps[:], ones_kj[:],
                                                 acc[h][:],
                                                 start=True, stop=True)
                                recip = fxpool.tile([1, NL], F32,
                                                    name=f"{x}rc{h}",
                                                    tag="recip", bufs=2)
                                nc.vector.reciprocal(out=recip[:],
                                                     in_=den_ps[:])
                                bc_sb = fxpool.tile([P, NL], F32,
                                                    name=f"{x}bcs{h}",
                                                    tag="bcs", bufs=2)
                                nc.gpsimd.partition_broadcast(bc_sb[:],
                                                              recip[:])
                                no = nopool.tile([P, NL], BF16,
                                                 name=f"{x}no{h}",
                                                 tag=f"no{h}")
                                nc.vector.tensor_tensor(
                                    out=no[:], in0=av_ps[h][:],
                                    in1=bc_sb[:],
                                    op=mybir.AluOpType.mult)
                                no_sb[h] = no

                        # partial out-projection over this wave's heads:
                        # fills the PE while ACT drains this wave's exps and
                        # the next collective lands.
                        for fi in range(NFI):
                            ps = pp.tile([P, NL], F32,
                                         name=f"{x}po{pair}_{fi}",
                                         tag="pso", bufs=1)
                            for h in heads:
                                nc.tensor.matmul(
                                    ps[:], wo_bf[fi][:, h * P:(h + 1) * P],
                                    no_sb[h][:],
                                    start=(h == heads[0]),
                                    stop=(h == heads[-1]))
                            oa = out_acc[:, fi * NL:(fi + 1) * NL]
                            if pair == 0:
                                nc.vector.tensor_copy(out=oa, in_=ps[:])
                            else:
                                o_sb = opool.tile([P, NL], BF16,
                                                  name=f"{x}o{fi}", tag="osb")
                                nc.vector.scalar_tensor_tensor(
                                    out=o_sb[:], in0=ps[:],
                                    scalar=bias_sb[:, fi:fi + 1],
                                    in1=oa,
                                    op0=mybir.AluOpType.add,
                                    op1=mybir.AluOpType.add)
                                nc.scalar.dma_start(out=outt_ext[fi],
                                                    in_=o_sb[:])

                if stop_after == "attn":
                    nc.sync.dma_start(out=outt_ext[0],
                                      in_=no_sb[0][:].bitcast(F32))
                    return

def build_program(reps=1, single_core=False):
    nc = bacc.Bacc("TRN2", target_bir_lowering=False, debug=False,
                   num_devices=1 if single_core else N_CORES)

    ext = (
        nc.dram_tensor("xt", [ND, P, NL], BF16,
                       kind="ExternalInput").ap(),
        nc.dram_tensor("wqtt", [4, ND, P, NL], BF16,
                       kind="ExternalInput").ap(),
        nc.dram_tensor("wkv01t", [ND, P, NL], BF16,
                       kind="ExternalInput").ap(),
        nc.dram_tensor("wkv23t", [ND, P, NL], BF16,
                       kind="ExternalInput").ap(),
        nc.dram_tensor("wott", [NFI, P, H * P], BF16,
                       kind="ExternalInput").ap(),
        nc.dram_tensor("biast", [P, NFI], F32, kind="ExternalInput").ap(),
        nc.dram_tensor("cost", [P, NL], F32, kind="ExternalInput").ap(),
        nc.dram_tensor("sint", [P, NL], F32, kind="ExternalInput").ap(),
        nc.dram_tensor("outt", [NFI, P, NL], BF16,
                       kind="ExternalOutput").ap(),
    )
    consts = (
        nc.inline_tensor(np.ones((P, 1), np.float16), name="ones_kj_c"),
    )

    with tile.TileContext(nc) as tc:
        for r in range(reps):
            # wait hints are absolute in the scheduler's clock: offset them
            # by the expected rep period so reps >= 1 keep the CC-gating
            # protection (otherwise stale hints are no-ops there).
            _emit(nc, tc, ext, consts, f"r{r}_" if reps > 1 else "",
                  single_core=single_core, t0=r * 0.34)

    nc.compile()
    return nc


def shard_inputs(x, cos, sin, wq, wkv, wo_w, wo_b):
    """Host-side prep: transpose/tile everything into DMA-friendly layouts."""
    x = np.asarray(x, np.float32)
    cos = np.asarray(cos, np.float32)
    sin = np.asarray(sin, np.float32)
    wq = np.asarray(wq, np.float32)
    wkv = np.asarray(wkv, np.float32)
    wo_w = np.asarray(wo_w, np.float32)
    wo_b = np.asarray(wo_b, np.float32)

    wqT = np.ascontiguousarray(wq.T)                      # [d, e]
    # tiles [hg, dt, 128, 512]
    wqtt = np.ascontiguousarray(
        wqT.reshape(ND, P, 4, NL).transpose(2, 0, 1, 3)).astype(
            ml_dtypes.bfloat16)
    wkvT = wkv.T                                          # [d, 1024]
    wk, wv = wkvT[:, 0:512], wkvT[:, 512:1024]
    # per pair: [d, 512] = [K pair (2*128) | V pair (2*128)]
    wkv01 = np.ascontiguousarray(
        np.concatenate([wk[:, 0:256], wv[:, 0:256]], axis=1)
    ).reshape(ND, P, NL).astype(ml_dtypes.bfloat16)
    wkv23 = np.ascontiguousarray(
        np.concatenate([wk[:, 256:512], wv[:, 256:512]], axis=1)
    ).reshape(ND, P, NL).astype(ml_dtypes.bfloat16)
    woT = wo_w.T                                          # [e, f]
    # [fi, a, h, b]: per fi a contiguous [128, 2048] block
    wott = np.ascontiguousarray(
        woT.reshape(H, P, NFI, P).transpose(2, 1, 0, 3)
    ).reshape(NFI, P, H * P).astype(ml_dtypes.bfloat16)
    biast = np.ascontiguousarray(wo_b.reshape(NFI, P).T)  # [128, 16] f32

    in_maps = []
    for c in range(N_CORES):
        b, blk = divmod(c, 4)
        r0 = blk * NL
        xt = np.ascontiguousarray(x[b, r0:r0 + NL, :].T).reshape(
            ND, P, NL).astype(ml_dtypes.bfloat16)
        cosT = cos[0, r0:r0 + NL, 0, :].T                 # [64, n]
        sinT = sin[0, r0:r0 + NL, 0, :].T
        cost = np.ascontiguousarray(np.vstack([cosT, cosT]))   # [128, n]
        sint = np.ascontiguousarray(np.vstack([sinT, sinT]))
        in_maps.append({
            "xt": xt, "wqtt": wqtt, "wkv01t": wkv01, "wkv23t": wkv23,
            "wott": wott, "biast": biast, "cost": cost, "sint": sint,
        })
    return in_maps


def assemble_output(results):
    out = np.empty((B, N, D), np.float32)
    for c in range(N_CORES):
        b, blk = divmod(c, 4)
        r0 = blk * NL
        # outt [NFI, P, NL] -> [d, n] -> transpose
        out[b, r0:r0 + NL, :] = results[c]["outt"].reshape(
            D, NL).astype(np.float32).T
    return out


def get_program(reps=1):
    key = ("nc", reps)
    if key not in _CACHE:
        _CACHE[key] = build_program(reps)
    return _CACHE[key]


def kernel(x, cos, sin, attn_mask, wq, wkv, wo_w, wo_b):
    # attn_mask is all-ones by construction (fill spec); ignored.
    nc = get_program()
    in_maps = shard_inputs(x, cos, sin, wq, wkv, wo_w, wo_b)
    res = bass_utils.run_bass_kernel_spmd(
        nc, in_maps, core_ids=list(range(N_CORES)))
    return assemble_output(res.results)
